# revision 17
# baseline (speedup 1.0000x reference)
"""DirGCNConv Trainium2 kernel (8 NeuronCores, SPMD).

Strategy (dest-sharded graph parallel):
  - Nodes are sharded into 8 contiguous destination ranges (one per core).
  - agg  [r] = out_is[r] * sum_{e: row=r} in_is[col_e]  * x[col_e]
  - agg_t[c] = in_is[c]  * sum_{e: col=c} out_is[row_e] * x[row_e]
  - Per core, per aggregation, edges are grouped by destination into a
    padded-CSR "slot" layout (slots padded to degree buckets L in LBUCKETS).
    Because indices for the hardware gather op are int16, each aggregation is
    split into two independent structures: sources < 32768 ("lo") and
    >= 32768 ("hi"); both partial sums are merged in the epilogue.
  - Device pipeline per slot-tile of 128 slots:
      dma_gather 512B rows of x  ->  scale by rsqrt(src degree)  ->
      TensorE matmul with a CONSTANT block-ones matrix reduces the L slots of
      each dest -> accumulate agg^T[96, dest] in PSUM -> transpose -> DRAM
      scratch (degree-bucket-permuted order).
  - Epilogue: dma_gather un-permutes scratch rows, merges lo+hi, applies the
    dest-side rsqrt scale, applies the two 96x96 linear layers + bias on
    TensorE, writes the output rows in natural order.
  All floating point math runs on device; the host only does integer index
  preprocessing (degree counts, sorting/bucketing, padding, layout packing).
"""

import math

import numpy as np

N = 50000
D = 96
NCORES = 8
ND = N // NCORES          # 6250 dests per core
HALF = 32768              # int16 index split
LBUCKETS = [2, 4, 6, 8, 10, 12, 14, 16, 20, 24, 32, 48, 64, 96, 128]
GROUP_DESTS = 512         # dest columns per PSUM group
CALL_TILES = 32           # slot-tiles per dma_gather call
EPI_CHUNK = 13            # out-tiles per epilogue gather call
NT_OUT = 52               # 52*128 = 6656 output rows (>= 6250)
SCRATCH_ROWS = 8192       # per-structure scratch rows (int16-addressable)
ALPHA = 0.5

_TILE_ND = {L: 128 // L for L in LBUCKETS}
_TILE_K = {L: (128 // L) * L for L in LBUCKETS}
_MCOL = {}
_off = 0
for _L in LBUCKETS:
    _MCOL[_L] = _off
    _off += _TILE_ND[_L]
MCOLS = _off  # 42


def _build_masters():
    m = np.zeros((128, MCOLS), np.float32)
    for L in LBUCKETS:
        nd = _TILE_ND[L]
        K = _TILE_K[L]
        for k in range(K):
            m[k, _MCOL[L] + k // L] = 1.0
    return m


def _wrap_idx(flat_idx):
    """int16 layout for dma_gather: [128, ni//16], wrapped by 16, replicated
    across the 8 Q7 core groups."""
    ni = flat_idx.shape[0]
    assert ni % 16 == 0
    w = flat_idx.reshape(ni // 16, 16).T.astype(np.int16)
    return np.tile(w, (8, 1))


class _Structure:
    """Host-side padded-CSR for one (aggregation, half) on one core."""

    def __init__(self, e_dest, e_src, sdeg_of_src):
        # sort edges by local dest
        order = np.argsort(e_dest, kind="stable")
        self.dsorted = e_dest[order]
        self.ssorted = e_src[order]
        self.sdeg = sdeg_of_src[order]      # src-side degree per edge
        self.cnt = np.bincount(e_dest, minlength=ND)
        self.starts = np.concatenate([[0], np.cumsum(self.cnt)])
        assert self.cnt.max(initial=0) <= 128, self.cnt.max()
        # bucket per dest (only dests with cnt>0 participate)
        self.buckets = {L: [] for L in LBUCKETS}
        for n in np.nonzero(self.cnt)[0]:
            c = self.cnt[n]
            for L in LBUCKETS:
                if L >= c:
                    self.buckets[L].append(int(n))
                    break

    def n_tiles(self, L):
        return math.ceil(len(self.buckets[L]) / _TILE_ND[L])


def _geometry(n_tiles_by_sL):
    """Uniform (cross-core) compile-time geometry: tiles, groups, calls."""
    geos = []
    for s in range(4):
        tiles = []  # (L, K, nd)
        for L in LBUCKETS:
            for _ in range(n_tiles_by_sL[s][L]):
                tiles.append((L, _TILE_K[L], _TILE_ND[L]))
        # pack into PSUM groups of <= GROUP_DESTS dest columns
        groups = []
        tmeta = []
        cur = 0
        for (L, K, nd) in tiles:
            if not groups or cur + nd > GROUP_DESTS:
                groups.append(0)
                cur = 0
            g = len(groups) - 1
            tmeta.append(dict(L=L, K=K, nd=nd, group=g, dcol=cur,
                              first=(cur == 0), last=False))
            cur += nd
            groups[g] = cur
        # mark last tile of each group
        seen = set()
        for t in reversed(tmeta):
            if t["group"] not in seen:
                t["last"] = True
                seen.add(t["group"])
        n_groups = len(groups)
        # guarantee a zero position: last group must have slack
        empty_group = False
        if n_groups == 0 or groups[-1] >= GROUP_DESTS:
            empty_group = True
            n_groups += 1
        pos_zero = (n_groups - 1) * GROUP_DESTS + GROUP_DESTS - 1
        assert n_groups * GROUP_DESTS <= SCRATCH_ROWS
        # gather calls
        calls = []
        t0 = 0
        while t0 < len(tmeta):
            c = min(CALL_TILES, len(tmeta) - t0)
            calls.append((t0, c))
            t0 += c
        geos.append(dict(tiles=tmeta, n_groups=n_groups,
                         empty_group=empty_group, pos_zero=pos_zero,
                         calls=calls))
    return geos


def _dest_positions(geo, structure):
    """Map local dest id -> scratch row for one structure (or pos_zero)."""
    pos = np.full(NT_OUT * 128, geo["pos_zero"], np.int64)
    ti = 0
    per_bucket = {L: 0 for L in LBUCKETS}
    for t in geo["tiles"]:
        L, nd = t["L"], t["nd"]
        dlist = structure.buckets[L]
        base = t["group"] * GROUP_DESTS + t["dcol"]
        for j in range(nd):
            di = per_bucket[L]
            if di < len(dlist):
                pos[dlist[di]] = base + j
                per_bucket[L] += 1
        ti += 1
    return pos


def _slot_arrays(geo, structure):
    """Per-slot source index (int16, within half) and src-degree (u8)."""
    ntiles = len(geo["tiles"])
    src = np.zeros((ntiles, 128), np.int64)     # pad -> row 0 (killed by deg 0)
    deg = np.zeros((ntiles, 128), np.uint8)
    per_bucket = {L: 0 for L in LBUCKETS}
    for ti, t in enumerate(geo["tiles"]):
        L, nd = t["L"], t["nd"]
        dlist = structure.buckets[L]
        for j in range(nd):
            di = per_bucket[L] + j
            if di >= len(dlist):
                continue
            n = dlist[di]
            c = structure.cnt[n]
            st = structure.starts[n]
            sl = slice(j * L, j * L + c)
            src[ti, sl] = structure.ssorted[st:st + c]
            deg[ti, sl] = np.minimum(structure.sdeg[st:st + c], 255)
        per_bucket[L] += nd
    return src, deg


def _build_program(geos, phases=("main", "epi")):
    import os
    nstruct = 4
    maxcalls = 10 ** 9
    import concourse.bass as bass
    import concourse.mybir as mybir
    import concourse.tile as tile
    from concourse import bacc

    f32 = mybir.dt.float32
    bf16 = mybir.dt.bfloat16
    i16 = mybir.dt.int16
    u8 = mybir.dt.uint8
    MULT = mybir.AluOpType.mult
    ADD = mybir.AluOpType.add

    total_tiles = [len(g["tiles"]) for g in geos]
    icol0 = []  # per structure: per call idx col offset in idx_main
    col = 0
    tcol0 = []
    tcol = 0
    for s, g in enumerate(geos):
        ic, tc_ = [], []
        for (t0, c) in g["calls"]:
            ic.append(col)
            tc_.append(tcol + t0)
            col += 8 * c
        icol0.append(ic)
        tcol0.append(tc_)
        tcol += total_tiles[s]
    IDX_COLS = col
    TCOLS = tcol

    nc = bacc.Bacc("TRN2", target_bir_lowering=False, debug=False,
                   num_swdge_queues=4)
    qctr = [0]

    def next_q():
        q = qctr[0] % 4
        qctr[0] += 1
        return q

    xpad = nc.dram_tensor("xpad", [N, 128], f32, kind="ExternalInput")
    idx_main = nc.dram_tensor("idx_main", [128, IDX_COLS], i16, kind="ExternalInput")
    deg_main = nc.dram_tensor("deg_main", [128, TCOLS], u8, kind="ExternalInput")
    idx_epi = nc.dram_tensor("idx_epi", [128, 8 * EPI_CHUNK * 4 * (NT_OUT // EPI_CHUNK)], i16, kind="ExternalInput")
    ddegA = nc.dram_tensor("ddegA", [128, NT_OUT], u8, kind="ExternalInput")
    ddegB = nc.dram_tensor("ddegB", [128, NT_OUT], u8, kind="ExternalInput")
    w_sd = nc.dram_tensor("w_sd", [96, 96], f32, kind="ExternalInput")
    w_ds = nc.dram_tensor("w_ds", [96, 96], f32, kind="ExternalInput")
    b_sd = nc.dram_tensor("b_sd", [96, 1], f32, kind="ExternalInput")
    b_ds = nc.dram_tensor("b_ds", [96, 1], f32, kind="ExternalInput")
    ident_d = nc.dram_tensor("ident", [128, 128], f32, kind="ExternalInput")
    masters_d = nc.dram_tensor("masters", [128, MCOLS], bf16, kind="ExternalInput")
    zeros_d = nc.dram_tensor("zeros32", [32, 96], f32, kind="ExternalInput")
    ones_d = nc.dram_tensor("ones32", [32, 512], f32, kind="ExternalInput")
    out_d = nc.dram_tensor("out", [NT_OUT * 128, 96], f32, kind="ExternalOutput")
    sr = max(g["n_groups"] for g in geos) * GROUP_DESTS
    scratch = nc.dram_tensor("scratch", [4, sr, 128], bf16,
                             kind="ExternalOutput")

    Copy = mybir.ActivationFunctionType.Copy
    Ident = mybir.ActivationFunctionType.Identity
    Sqrt = mybir.ActivationFunctionType.Sqrt

    def val_from_deg(nc, pool, deg_t, c, zb, pfx=""):
        """fp32 rsqrt(deg) masked to 0 where deg==0, from u8 degrees."""
        degf = pool.tile([128, c], f32, tag=pfx + "degf")
        nc.vector.tensor_copy(degf[:], deg_t[:])
        vmask = pool.tile([128, c], f32, tag=pfx + "vmask")
        nc.vector.tensor_scalar_min(vmask[:], degf[:], 1.0)
        vmax = pool.tile([128, c], f32, tag=pfx + "vmax")
        nc.vector.tensor_scalar_max(vmax[:], degf[:], 1.0)
        vrec = pool.tile([128, c], f32, tag=pfx + "vrec")
        nc.vector.reciprocal(vrec[:], vmax[:])
        vsq = pool.tile([128, c], f32, tag=pfx + "vsq")
        nc.scalar.activation(vsq[:], vrec[:], Sqrt, bias=zb[:, 0:1])
        val = pool.tile([128, c], f32, tag=pfx + "val")
        nc.vector.tensor_tensor(val[:], vsq[:], vmask[:], op=MULT)
        return val

    with tile.TileContext(nc) as tc:
        with (
            tc.tile_pool(name="const", bufs=1) as cpool,
            tc.tile_pool(name="meta", bufs=3) as mpool,
            tc.tile_pool(name="g", bufs=4) as gpool,
            tc.tile_pool(name="gs", bufs=3) as spool,
            tc.tile_pool(name="epi", bufs=4) as epool,
            tc.tile_pool(name="fin", bufs=3) as fpool,
            tc.tile_pool(name="psum", bufs=2, space="PSUM") as ppool,
            tc.tile_pool(name="psumt", bufs=4, space="PSUM") as ptpool,
            tc.tile_pool(name="psumo", bufs=2, space="PSUM") as popool,
        ):
            ident = cpool.tile([128, 128], f32)
            nc.sync.dma_start(out=ident[:], in_=ident_d[:, :])
            masters = cpool.tile([128, MCOLS], bf16)
            nc.sync.dma_start(out=masters[:], in_=masters_d[:, :])
            zeros32 = cpool.tile([32, 96], f32)
            nc.sync.dma_start(out=zeros32[:], in_=zeros_d[:, :])
            ones32 = cpool.tile([32, 512], f32)
            nc.sync.dma_start(out=ones32[:], in_=ones_d[:, :])
            zb = cpool.tile([128, 1], f32, tag="zb")
            nc.vector.memset(zb[:], 0.0)
            idx_all = cpool.tile([128, IDX_COLS], i16, tag="idx_all")
            nc.sync.dma_start(out=idx_all[:], in_=idx_main[:, :])
            deg_all = cpool.tile([128, TCOLS], u8, tag="deg_all")
            nc.sync.dma_start(out=deg_all[:], in_=deg_main[:, :])
            epi_all = cpool.tile(
                [128, 8 * EPI_CHUNK * 4 * (NT_OUT // EPI_CHUNK)], i16,
                tag="epi_all")
            nc.sync.dma_start(out=epi_all[:], in_=idx_epi[:, :])
            val_all = val_from_deg(nc, cpool, deg_all, TCOLS, zb, pfx="va")

            def finalize_group(s, g, psum):
                aggT = fpool.tile([96, 512], f32, tag="aggT")
                nc.scalar.copy(aggT[:], psum[:])
                for k in range(4):
                    pt = ptpool.tile([128, 96], f32, tag="pt")
                    nc.tensor.transpose(pt[:], aggT[:, k * 128:(k + 1) * 128],
                                        ident[0:96, 0:96])
                    rows = fpool.tile([128, 128], bf16, tag="rows")
                    nc.vector.tensor_copy(rows[:, 0:96], pt[:])
                    nc.sync.dma_start(
                        out=scratch[s, g * GROUP_DESTS + k * 128:
                                    g * GROUP_DESTS + (k + 1) * 128, :],
                        in_=rows[:])

            rep_main = int(os.environ.get("REPEAT_MAIN", "1"))
            skip_mm = bool(int(os.environ.get("SKIP_MM", "0")))
            skip_gather = bool(int(os.environ.get("SKIP_GATHER", "0")))
            struct_list = [s for _ in range(rep_main)
                           for s in range(nstruct if "main" in phases else 0)]
            for s in struct_list:
                geo = geos[s]
                half = s % 2
                xview = xpad[0:HALF, :] if half == 0 else xpad[HALF:N, :]
                psum = None
                for ci, (t0, c) in enumerate(geo["calls"][:maxcalls]):
                    ni = 128 * c
                    idx_t = idx_all[:, icol0[s][ci]:icol0[s][ci] + 8 * c]
                    G = gpool.tile([128, c, 128], f32, tag="G")
                    if not skip_gather:
                        nc.gpsimd.dma_gather(
                            out_ap=G[:], in_ap=xview, idxs_ap=idx_t[:],
                            num_idxs=ni, num_idxs_reg=ni, elem_size=128,
                            single_packet=False, queue_num=next_q())
                    else:
                        nc.sync.dma_start(
                            out=G[:],
                            in_=xpad[0:ni, :].rearrange("(c p) d -> p c d", p=128))
                    val = val_all[:, tcol0[s][ci]:tcol0[s][ci] + c]
                    Gs = spool.tile([128, c, 96], bf16, tag="Gs")
                    nc.vector.tensor_tensor(
                        Gs[:], G[:, :, 0:96],
                        val[:, :, None].to_broadcast([128, c, 96]), op=MULT)
                    for j in range(c if not skip_mm else 0):
                        t = geo["tiles"][t0 + j]
                        if t["first"]:
                            psum = ppool.tile([96, 512], f32, tag="acc")
                            nc.tensor.matmul(psum[:], zeros32[:], ones32[:],
                                             start=True, stop=False)
                        K, nd, L = t["K"], t["nd"], t["L"]
                        nc.tensor.matmul(
                            psum[:, t["dcol"]:t["dcol"] + nd],
                            lhsT=Gs[0:K, j, :],
                            rhs=masters[0:K, _MCOL[L]:_MCOL[L] + nd],
                            start=False, stop=t["last"])
                        if t["last"]:
                            finalize_group(s, t["group"], psum)
                if geo["empty_group"]:
                    psum = ppool.tile([96, 512], f32, tag="acc")
                    nc.tensor.matmul(psum[:], zeros32[:], ones32[:],
                                     start=True, stop=True)
                    finalize_group(s, geo["n_groups"] - 1, psum)

            tc.strict_bb_all_engine_barrier()

            # ---- epilogue ----
            # W lhsT prep: lhsT[f, o] = W[o, f]^T, alpha-scaled
            wts = []
            if "epi" in phases:
                for wsrc, a in ((w_sd, ALPHA), (w_ds, 1.0 - ALPHA)):
                    w_sb = cpool.tile([96, 96], f32, tag="wld")
                    nc.sync.dma_start(out=w_sb[:], in_=wsrc[:, :])
                    w_sc = cpool.tile([96, 96], f32, tag="wsc")
                    nc.vector.tensor_scalar_mul(w_sc[:], w_sb[:], float(a))
                    pw = ptpool.tile([96, 96], f32, tag="pt")
                    nc.tensor.transpose(pw[:], w_sc[:], ident[0:96, 0:96])
                    wT = cpool.tile([96, 96], f32, tag=f"wT{len(wts)}")
                    nc.vector.tensor_copy(wT[:], pw[:])
                    wts.append(wT)
                bsd_sb = cpool.tile([96, 1], f32, tag="bsd")
                nc.sync.dma_start(out=bsd_sb[:], in_=b_sd[:, :])
                bds_sb = cpool.tile([96, 1], f32, tag="bds")
                nc.sync.dma_start(out=bds_sb[:], in_=b_ds[:, :])
                bcomb = cpool.tile([96, 1], f32, tag="bcomb")
                nc.vector.scalar_tensor_tensor(
                    bcomb[:], bsd_sb[:], float(ALPHA / (1.0 - ALPHA)), bds_sb[:],
                    op0=MULT, op1=ADD)
                nc.vector.tensor_scalar_mul(bcomb[:], bcomb[:], float(1.0 - ALPHA))
                # dest scales
                dscs = []
                for dd in (ddegA, ddegB):
                    dd_t = cpool.tile([128, NT_OUT], u8, tag="ddu8")
                    nc.sync.dma_start(out=dd_t[:], in_=dd[:, :])
                    dsc = val_from_deg(nc, cpool, dd_t, NT_OUT, zb,
                                       pfx=f"d{len(dscs)}")
                    dscs.append(dsc)

            rep_epi = int(os.environ.get("REPEAT_EPI", "1"))
            for chunk_rep in range(rep_epi * (NT_OUT // EPI_CHUNK)
                                   if "epi" in phases else 0):
                chunk = chunk_rep % (NT_OUT // EPI_CHUNK)
                gE = []
                for s in range(4):
                    ecall = chunk * 4 + s  # not used; offset computed below
                    coloff = (chunk * 4 + s) * 8 * EPI_CHUNK
                    eidx = epi_all[:, coloff:coloff + 8 * EPI_CHUNK]
                    ge = epool.tile([128, EPI_CHUNK, 128], bf16, tag="ge")
                    nc.gpsimd.dma_gather(
                        out_ap=ge[:], in_ap=scratch[s], idxs_ap=eidx,
                        num_idxs=128 * EPI_CHUNK, num_idxs_reg=128 * EPI_CHUNK,
                        elem_size=128, single_packet=False,
                        queue_num=next_q())
                    gE.append(ge)
                for tl in range(EPI_CHUNK):
                    t = chunk * EPI_CHUNK + tl
                    rhss = []
                    for a in range(2):
                        r = fpool.tile([128, 96], f32, tag="rmerge")
                        nc.vector.tensor_tensor(
                            r[:], gE[2 * a][:, tl, 0:96], gE[2 * a + 1][:, tl, 0:96],
                            op=ADD)
                        rs = fpool.tile([128, 96], f32, tag="rscale")
                        nc.scalar.activation(rs[:], r[:], Copy,
                                             scale=dscs[a][:, t:t + 1])
                        pT = ptpool.tile([96, 128], f32, tag="pt")
                        nc.tensor.transpose(pT[:], rs[:], ident[:, :])
                        rhs = fpool.tile([96, 128], f32, tag="rhs")
                        nc.vector.tensor_copy(rhs[:], pT[:])
                        rhss.append(rhs)
                    po = popool.tile([96, 128], f32, tag="po")
                    nc.tensor.matmul(po[:], wts[0][:], rhss[0][:],
                                     start=True, stop=False)
                    nc.tensor.matmul(po[:], wts[1][:], rhss[1][:],
                                     start=False, stop=True)
                    outT = fpool.tile([96, 128], f32, tag="outT")
                    nc.scalar.activation(outT[:], po[:], Ident, bias=bcomb[:, 0:1])
                    pf = ptpool.tile([128, 96], f32, tag="pt")
                    nc.tensor.transpose(pf[:], outT[:], ident[0:96, 0:96])
                    orow = fpool.tile([128, 96], f32, tag="orow")
                    nc.vector.tensor_copy(orow[:], pf[:])
                    nc.sync.dma_start(out=out_d[t * 128:(t + 1) * 128, :],
                                      in_=orow[:])

    nc.compile()
    return nc


def _host_prep(x, edge_index):
    row = np.asarray(edge_index[0], np.int64)
    col = np.asarray(edge_index[1], np.int64)
    out_deg = np.bincount(row, minlength=N)
    in_deg = np.bincount(col, minlength=N)

    # per-core structures: s=0 agg-lo, 1 agg-hi, 2 aggt-lo, 3 aggt-hi
    structs = []
    for d in range(NCORES):
        lo, hi = d * ND, (d + 1) * ND
        per_core = []
        for a in range(2):
            dest = row if a == 0 else col
            src = col if a == 0 else row
            sdeg = in_deg if a == 0 else out_deg
            core_mask = (dest >= lo) & (dest < hi)
            for h in range(2):
                m = core_mask & ((src < HALF) if h == 0 else (src >= HALF))
                per_core.append(_Structure(
                    dest[m] - lo, src[m] - h * HALF, sdeg[src[m]]))
        structs.append(per_core)

    n_tiles_by_sL = []
    for s in range(4):
        n_tiles_by_sL.append(
            {L: max(structs[d][s].n_tiles(L) for d in range(NCORES))
             for L in LBUCKETS})
    geos = _geometry(n_tiles_by_sL)

    # shared constants
    xpad = np.zeros((N, 128), np.float32)
    xpad[:, :D] = x
    import ml_dtypes
    consts = dict(
        xpad=xpad,
        ident=np.eye(128, dtype=np.float32),
        masters=_build_masters().astype(ml_dtypes.bfloat16),
        zeros32=np.zeros((32, 96), np.float32),
        ones32=np.ones((32, 512), np.float32),
    )

    in_maps = []
    for d in range(NCORES):
        idx_cols, deg_cols = [], []
        for s in range(4):
            geo = geos[s]
            src, deg = _slot_arrays(geo, structs[d][s])
            deg_cols.append(deg.T)                      # [128, ntiles]
            for (t0, c) in geo["calls"]:
                flat = src[t0:t0 + c].reshape(-1)       # [128c]
                idx_cols.append(_wrap_idx(flat))
        idx_main = np.concatenate(idx_cols, axis=1)
        deg_main = np.concatenate(deg_cols, axis=1).astype(np.uint8)

        epi_cols = []
        poss = [_dest_positions(geos[s], structs[d][s]) for s in range(4)]
        for chunk in range(NT_OUT // EPI_CHUNK):
            for s in range(4):
                rows = np.arange(chunk * EPI_CHUNK * 128,
                                 (chunk + 1) * EPI_CHUNK * 128)
                epi_cols.append(_wrap_idx(poss[s][rows]))
        idx_epi = np.concatenate(epi_cols, axis=1)

        dd = []
        for degarr in (out_deg, in_deg):
            v = np.zeros(NT_OUT * 128, np.int64)
            v[:ND] = degarr[d * ND:(d + 1) * ND]
            dd.append(np.minimum(v, 255).astype(np.uint8).reshape(NT_OUT, 128).T)

        in_maps.append(dict(
            consts,
            idx_main=idx_main.astype(np.int16),
            deg_main=deg_main,
            idx_epi=idx_epi.astype(np.int16),
            ddegA=dd[0], ddegB=dd[1],
        ))
    return geos, in_maps


_CACHE = {}


def kernel(x, W_sd, b_sd, W_ds, b_ds, edge_index):
    from concourse.bass_utils import run_bass_kernel_spmd

    x = np.asarray(x, np.float32)
    geos, in_maps = _host_prep(x, edge_index)
    wmats = dict(
        w_sd=np.asarray(W_sd, np.float32),
        w_ds=np.asarray(W_ds, np.float32),
        b_sd=np.asarray(b_sd, np.float32).reshape(96, 1),
        b_ds=np.asarray(b_ds, np.float32).reshape(96, 1),
    )
    for m in in_maps:
        m.update(wmats)

    key = "prog"
    if key not in _CACHE:
        _CACHE[key] = _build_program(geos)
    nc = _CACHE[key]

    res = run_bass_kernel_spmd(nc, in_maps, core_ids=list(range(NCORES)))
    out = np.zeros((N, D), np.float32)
    for d in range(NCORES):
        out[d * ND:(d + 1) * ND] = res.results[d]["out"][:ND]
    return out

